# revision 1
# baseline (speedup 1.0000x reference)
"""Trainium2 kernel for nn_ActorCritic (GNN message passing + edge-wise actor MLP).

Strategy (per sharding hint): edges sharded across 8 NeuronCores. The
dominant compute -- the per-edge actor MLP (3.2M edges x [4->64->64->1]) --
runs on device as a 2-block-packed PE matmul pipeline (edges stream on the
free dim, two 64-wide blocks packed into the 128-wide PE array). Node
features and the tiny MLP weights are replicated. Graph index prep /
sharding / unshard run on host.
"""

import os
import sys

import numpy as np

sys.path.insert(0, "/opt/trn_rl_repo")

N_NODES = 100000
N_EDGES = 3200000
BN_EPS = 1e-5
N_CORES = 8
F = 512  # matmul free-dim tile (fp32 PSUM bank limit)
EDGES_PER_CORE = N_EDGES // N_CORES  # 400000
NT = (EDGES_PER_CORE + 2 * F - 1) // (2 * F)  # 391 iterations, 1024 edges each
EC_PAD = NT * 2 * F

_NC_CACHE = {}


def _np32(a):
    return np.asarray(a, dtype=np.float32)


def _segment_sum(vals, idx, n):
    # vals [E, 2] f32, idx [E] int64 -> [n, 2] f32
    out = np.empty((n, vals.shape[1]), dtype=np.float32)
    for f in range(vals.shape[1]):
        out[:, f] = np.bincount(idx, weights=vals[:, f], minlength=n).astype(
            np.float32
        )
    return out


def _mlp_host(p, h, bn):
    Ws = [_np32(w) for w in p["W"]]
    bs = [_np32(b) for b in p["b"]]
    n = len(Ws)
    for i in range(n - 1):
        h = h @ Ws[i] + bs[i]
        if bn:
            m = h.mean(axis=0)
            v = h.var(axis=0)
            h = (h - m) / np.sqrt(v + BN_EPS) * _np32(p["gamma"][i]) + _np32(
                p["beta"][i]
            )
        h = np.maximum(h, 0.0)
    return h @ Ws[-1] + bs[-1]


def _build_nc():
    """Build + compile the 8-core SPMD actor-MLP kernel (cached per process)."""
    if "nc" in _NC_CACHE:
        return _NC_CACHE["nc"]
    import concourse.bacc as bacc
    import concourse.tile as tile
    from concourse import bass, mybir

    f32 = mybir.dt.float32
    nc = bacc.Bacc("TRN2", target_bir_lowering=False, debug=False,
                   num_devices=N_CORES)
    pt = nc.dram_tensor("pt", [8, NT * F], f32, kind="ExternalInput")
    w1 = nc.dram_tensor("w1", [8, 128], f32, kind="ExternalInput")
    b1 = nc.dram_tensor("b1", [128, 1], f32, kind="ExternalInput")
    w2 = nc.dram_tensor("w2", [128, 128], f32, kind="ExternalInput")
    b2 = nc.dram_tensor("b2", [128, 1], f32, kind="ExternalInput")
    w3 = nc.dram_tensor("w3", [128, 2], f32, kind="ExternalInput")
    lg = nc.dram_tensor("lg", [2, NT * F], f32, kind="ExternalOutput")

    CH = 8  # tiles per DMA chunk
    with tile.TileContext(nc) as tc:
        with (
            tc.tile_pool(name="const", bufs=1) as cpool,
            tc.tile_pool(name="io", bufs=3) as iopool,
            tc.tile_pool(name="act", bufs=3) as apool,
            tc.tile_pool(name="ps1", bufs=2, space="PSUM") as ps1pool,
            tc.tile_pool(name="ps2", bufs=2, space="PSUM") as ps2pool,
            tc.tile_pool(name="ps3", bufs=2, space="PSUM") as ps3pool,
        ):
            w1t = cpool.tile([8, 128], f32)
            nc.sync.dma_start(out=w1t[:], in_=w1.ap())
            b1t = cpool.tile([128, 1], f32)
            nc.sync.dma_start(out=b1t[:], in_=b1.ap())
            w2t = cpool.tile([128, 128], f32)
            nc.sync.dma_start(out=w2t[:], in_=w2.ap())
            b2t = cpool.tile([128, 1], f32)
            nc.sync.dma_start(out=b2t[:], in_=b2.ap())
            w3t = cpool.tile([128, 2], f32)
            nc.sync.dma_start(out=w3t[:], in_=w3.ap())

            for j0 in range(0, NT, CH):
                n = min(CH, NT - j0)
                rhs = iopool.tile([8, CH * F], f32, tag="rhs")
                nc.sync.dma_start(
                    out=rhs[:, : n * F], in_=pt.ap()[:, j0 * F : (j0 + n) * F]
                )
                lgt = iopool.tile([2, CH * F], f32, tag="lgt")
                for t in range(n):
                    r = rhs[:, t * F : (t + 1) * F]
                    ps1 = ps1pool.tile([128, F], f32, tag="ps1")
                    nc.tensor.matmul(ps1[:], w1t[:], r, start=True, stop=True)
                    h1 = apool.tile([128, F], f32, tag="h1")
                    # h1 = max(ps1 + b1, 0)  (bias-add + relu fused on DVE)
                    nc.vector.tensor_scalar(
                        out=h1[:], in0=ps1[:],
                        scalar1=b1t[:, 0:1], scalar2=0.0,
                        op0=mybir.AluOpType.add, op1=mybir.AluOpType.max,
                    )
                    ps2 = ps2pool.tile([128, F], f32, tag="ps2")
                    nc.tensor.matmul(ps2[:], w2t[:], h1[:], start=True, stop=True)
                    h2 = apool.tile([128, F], f32, tag="h2")
                    nc.scalar.activation(
                        h2[:], ps2[:], mybir.ActivationFunctionType.Relu,
                        bias=b2t[:, 0:1], scale=1.0,
                    )
                    ps3 = ps3pool.tile([2, F], f32, tag="ps3")
                    nc.tensor.matmul(ps3[:], w3t[:], h2[:], start=True, stop=True)
                    nc.vector.tensor_copy(lgt[:, t * F : (t + 1) * F], ps3[:])
                nc.sync.dma_start(
                    out=lg.ap()[:, j0 * F : (j0 + n) * F], in_=lgt[:, : n * F]
                )
    nc.compile()
    _NC_CACHE["nc"] = nc
    return nc


def kernel(x, edge_index, gin_params, actor_params, critic_params):
    from concourse import bass_utils

    x = _np32(x)
    ei = np.asarray(edge_index)
    idx_dtype = ei.dtype
    ei = ei.astype(np.int64)
    src = np.concatenate([ei[0], ei[1]])
    dst = np.concatenate([ei[1], ei[0]])

    # ---- GIN stack (tiny 2->3->3->2 MLPs; graph structure prep on host) ----
    feats = x
    for p in gin_params:
        agg = _segment_sum(feats[src], dst, N_NODES)
        feats = _mlp_host(p, feats + agg, bn=True)

    graph_emb = feats.mean(axis=0, keepdims=True).astype(np.float32)  # (1, 2)
    value = _mlp_host(critic_params, graph_emb, bn=False)  # (1, 1)

    # ---- actor MLP on device: logits[e] = MLP(state ++ n1 ++ n2) ----
    W1 = _np32(actor_params["W"][0])  # (6, 64)
    W2 = _np32(actor_params["W"][1])  # (64, 64)
    W3 = _np32(actor_params["W"][2])  # (64, 1)
    b1 = _np32(actor_params["b"][0])
    b2 = _np32(actor_params["b"][1])
    b3 = _np32(actor_params["b"][2])  # constant shift -> softmax-invariant

    W1eff = np.ascontiguousarray(W1[2:6])  # (4, 64)
    b1c = b1 + graph_emb[0] @ W1[0:2]  # (64,)

    # block-diagonal 2x packing into the 128-wide PE array
    W1s = np.zeros((8, 128), np.float32)
    W1s[0:4, 0:64] = W1eff
    W1s[4:8, 64:128] = W1eff
    W2s = np.zeros((128, 128), np.float32)
    W2s[0:64, 0:64] = W2
    W2s[64:128, 64:128] = W2
    w3s = np.zeros((128, 2), np.float32)
    w3s[0:64, 0:1] = W3
    w3s[64:128, 1:2] = W3
    b1s = np.concatenate([b1c, b1c]).reshape(128, 1).astype(np.float32)
    b2s = np.concatenate([b2, b2]).reshape(128, 1).astype(np.float32)

    n1 = feats[ei[0]]  # (E, 2)
    n2 = feats[ei[1]]
    PT = np.ascontiguousarray(
        np.concatenate([n1, n2], axis=1).T
    )  # (4, E) rows: n1f0,n1f1,n2f0,n2f1

    nc = _build_nc()
    in_maps = []
    for c in range(N_CORES):
        ptc = PT[:, c * EDGES_PER_CORE : (c + 1) * EDGES_PER_CORE]
        if EC_PAD != EDGES_PER_CORE:
            ptc = np.concatenate(
                [ptc, np.zeros((4, EC_PAD - EDGES_PER_CORE), np.float32)], axis=1
            )
        # pack: [4, NT, 2, F] -> [2,4,NT,F] -> [8, NT*F]
        pts = np.ascontiguousarray(
            ptc.reshape(4, NT, 2, F).transpose(2, 0, 1, 3).reshape(8, NT * F)
        )
        in_maps.append(
            {"pt": pts, "w1": W1s, "b1": b1s, "w2": W2s, "b2": b2s, "w3": w3s}
        )

    trace = bool(int(os.environ.get("KBENCH_TRACE", "0")))
    res = bass_utils.run_bass_kernel_spmd(
        nc, in_maps, core_ids=list(range(N_CORES)), trace=trace
    )
    if trace:
        _NC_CACHE["last_exec_time_ns"] = res.exec_time_ns

    logits = np.empty(N_EDGES, np.float32)
    for c in range(N_CORES):
        lgc = res.results[c]["lg"].reshape(2, NT, F).transpose(1, 0, 2).ravel()
        logits[c * EDGES_PER_CORE : (c + 1) * EDGES_PER_CORE] = lgc[
            :EDGES_PER_CORE
        ]
    logits = logits + b3[0]

    # softmax over all edges (normalization on host)
    m = logits.max()
    e = np.exp((logits - m).astype(np.float64))
    pi = (e / e.sum()).astype(np.float32).reshape(N_EDGES, 1)
    return pi, value.astype(np.float32)


if __name__ == "__main__":
    # smoke test with random data
    rng = np.random.default_rng(0)
    x = rng.standard_normal((N_NODES, 2), dtype=np.float32)
    ei = rng.integers(0, N_NODES, size=(2, N_EDGES), dtype=np.int64)

    def mk_mlp(dims, bn):
        p = {
            "W": [rng.standard_normal((a, b), dtype=np.float32) / np.sqrt(a)
                  for a, b in zip(dims[:-1], dims[1:])],
            "b": [np.zeros(b, np.float32) for b in dims[1:]],
        }
        if bn:
            p["gamma"] = [np.ones(d, np.float32) for d in dims[1:-1]]
            p["beta"] = [np.zeros(d, np.float32) for d in dims[1:-1]]
        return p

    gin = [mk_mlp([2, 3, 3, 2], True) for _ in range(3)]
    actor = mk_mlp([6, 64, 64, 1], False)
    critic = mk_mlp([2, 64, 64, 1], False)
    pi, v = kernel(x=x, edge_index=ei, gin_params=gin, actor_params=actor,
                   critic_params=critic)
    print("pi", pi.shape, pi.sum(), "value", v)


# revision 3
# speedup vs baseline: 35.4260x; 35.4260x over previous
"""Trainium2 kernel for nn_ActorCritic (GNN message passing + edge-wise actor MLP).

Strategy (per sharding hint): edges sharded across 8 NeuronCores. The
dominant compute -- the per-edge actor MLP (3.2M edges x [4->64->64->1]) --
runs on device as a 2-block-packed PE matmul pipeline (edges stream on the
free dim, two 64-wide blocks packed into the 128-wide PE array). Node
features and the tiny MLP weights are replicated. Graph index prep /
sharding / unshard run on host.
"""

import os
import sys

import numpy as np

sys.path.insert(0, "/opt/trn_rl_repo")

N_NODES = 100000
N_EDGES = 3200000
BN_EPS = 1e-5
N_CORES = 8
F = 512  # matmul free-dim tile (fp32 PSUM bank limit)
EDGES_PER_CORE = N_EDGES // N_CORES  # 400000
NT = (EDGES_PER_CORE + 2 * F - 1) // (2 * F)  # 391 iterations, 1024 edges each
EC_PAD = NT * 2 * F

_NC_CACHE = {}


def _np32(a):
    return np.asarray(a, dtype=np.float32)


def _segment_sum(vals, idx, n):
    # vals [E, 2] f32, idx [E] int64 -> [n, 2] f32
    out = np.empty((n, vals.shape[1]), dtype=np.float32)
    for f in range(vals.shape[1]):
        out[:, f] = np.bincount(idx, weights=vals[:, f], minlength=n).astype(
            np.float32
        )
    return out


def _mlp_host(p, h, bn):
    Ws = [_np32(w) for w in p["W"]]
    bs = [_np32(b) for b in p["b"]]
    n = len(Ws)
    for i in range(n - 1):
        h = h @ Ws[i] + bs[i]
        if bn:
            m = h.mean(axis=0)
            v = h.var(axis=0)
            h = (h - m) / np.sqrt(v + BN_EPS) * _np32(p["gamma"][i]) + _np32(
                p["beta"][i]
            )
        h = np.maximum(h, 0.0)
    return h @ Ws[-1] + bs[-1]


def _build_nc():
    """Build + compile the 8-core SPMD actor-MLP kernel (cached per process)."""
    if "nc" in _NC_CACHE:
        return _NC_CACHE["nc"]
    import concourse.bacc as bacc
    import concourse.tile as tile
    from concourse import bass, mybir

    f32 = mybir.dt.float32
    nc = bacc.Bacc("TRN2", target_bir_lowering=False, debug=False,
                   num_devices=N_CORES)
    pt = nc.dram_tensor("pt", [8, NT * F], f32, kind="ExternalInput")
    w1 = nc.dram_tensor("w1", [8, 128], f32, kind="ExternalInput")
    b1 = nc.dram_tensor("b1", [128, 1], f32, kind="ExternalInput")
    w2 = nc.dram_tensor("w2", [128, 128], f32, kind="ExternalInput")
    b2 = nc.dram_tensor("b2", [128, 1], f32, kind="ExternalInput")
    w3 = nc.dram_tensor("w3", [128, 2], f32, kind="ExternalInput")
    lg = nc.dram_tensor("lg", [2, NT * F], f32, kind="ExternalOutput")

    CH = 8  # tiles per DMA chunk
    with tile.TileContext(nc) as tc:
        with (
            tc.tile_pool(name="const", bufs=1) as cpool,
            tc.tile_pool(name="io", bufs=3) as iopool,
            tc.tile_pool(name="act", bufs=3) as apool,
            tc.tile_pool(name="ps1", bufs=2, space="PSUM") as ps1pool,
            tc.tile_pool(name="ps2", bufs=2, space="PSUM") as ps2pool,
            tc.tile_pool(name="ps3", bufs=2, space="PSUM") as ps3pool,
        ):
            w1t = cpool.tile([8, 128], f32)
            nc.sync.dma_start(out=w1t[:], in_=w1.ap())
            b1t = cpool.tile([128, 1], f32)
            nc.sync.dma_start(out=b1t[:], in_=b1.ap())
            w2t = cpool.tile([128, 128], f32)
            nc.sync.dma_start(out=w2t[:], in_=w2.ap())
            b2t = cpool.tile([128, 1], f32)
            nc.sync.dma_start(out=b2t[:], in_=b2.ap())
            w3t = cpool.tile([128, 2], f32)
            nc.sync.dma_start(out=w3t[:], in_=w3.ap())

            for j0 in range(0, NT, CH):
                n = min(CH, NT - j0)
                rhs = iopool.tile([8, CH * F], f32, tag="rhs")
                nc.sync.dma_start(
                    out=rhs[:, : n * F], in_=pt.ap()[:, j0 * F : (j0 + n) * F]
                )
                lgt = iopool.tile([2, CH * F], f32, tag="lgt")
                for t in range(n):
                    r = rhs[:, t * F : (t + 1) * F]
                    ps1 = ps1pool.tile([128, F], f32, tag="ps1")
                    nc.tensor.matmul(ps1[:], w1t[:], r, start=True, stop=True)
                    h1 = apool.tile([128, F], f32, tag="h1")
                    # h1 = max(ps1 + b1, 0)  (bias-add + relu fused on DVE)
                    nc.vector.tensor_scalar(
                        out=h1[:], in0=ps1[:],
                        scalar1=b1t[:, 0:1], scalar2=0.0,
                        op0=mybir.AluOpType.add, op1=mybir.AluOpType.max,
                    )
                    ps2 = ps2pool.tile([128, F], f32, tag="ps2")
                    nc.tensor.matmul(ps2[:], w2t[:], h1[:], start=True, stop=True)
                    h2 = apool.tile([128, F], f32, tag="h2")
                    nc.scalar.activation(
                        h2[:], ps2[:], mybir.ActivationFunctionType.Relu,
                        bias=b2t[:, 0:1], scale=1.0,
                    )
                    ps3 = ps3pool.tile([2, F], f32, tag="ps3")
                    nc.tensor.matmul(ps3[:], w3t[:], h2[:], start=True, stop=True)
                    # alternate the PSUM->SBUF logit copy between ACT and DVE
                    # so neither engine becomes the per-iteration bottleneck
                    if t % 2 == 0:
                        nc.scalar.copy(lgt[:, t * F : (t + 1) * F], ps3[:])
                    else:
                        nc.vector.tensor_copy(lgt[:, t * F : (t + 1) * F], ps3[:])
                nc.sync.dma_start(
                    out=lg.ap()[:, j0 * F : (j0 + n) * F], in_=lgt[:, : n * F]
                )
    nc.compile()
    _NC_CACHE["nc"] = nc
    return nc


def kernel(x, edge_index, gin_params, actor_params, critic_params):
    from concourse import bass_utils

    x = _np32(x)
    ei = np.asarray(edge_index)
    idx_dtype = ei.dtype
    ei = ei.astype(np.int64)
    src = np.concatenate([ei[0], ei[1]])
    dst = np.concatenate([ei[1], ei[0]])

    # ---- GIN stack (tiny 2->3->3->2 MLPs; graph structure prep on host) ----
    feats = x
    for p in gin_params:
        agg = _segment_sum(feats[src], dst, N_NODES)
        feats = _mlp_host(p, feats + agg, bn=True)

    graph_emb = feats.mean(axis=0, keepdims=True).astype(np.float32)  # (1, 2)
    value = _mlp_host(critic_params, graph_emb, bn=False)  # (1, 1)

    # ---- actor MLP on device: logits[e] = MLP(state ++ n1 ++ n2) ----
    W1 = _np32(actor_params["W"][0])  # (6, 64)
    W2 = _np32(actor_params["W"][1])  # (64, 64)
    W3 = _np32(actor_params["W"][2])  # (64, 1)
    b1 = _np32(actor_params["b"][0])
    b2 = _np32(actor_params["b"][1])
    b3 = _np32(actor_params["b"][2])  # constant shift -> softmax-invariant

    W1eff = np.ascontiguousarray(W1[2:6])  # (4, 64)
    b1c = b1 + graph_emb[0] @ W1[0:2]  # (64,)

    # block-diagonal 2x packing into the 128-wide PE array
    W1s = np.zeros((8, 128), np.float32)
    W1s[0:4, 0:64] = W1eff
    W1s[4:8, 64:128] = W1eff
    W2s = np.zeros((128, 128), np.float32)
    W2s[0:64, 0:64] = W2
    W2s[64:128, 64:128] = W2
    w3s = np.zeros((128, 2), np.float32)
    w3s[0:64, 0:1] = W3
    w3s[64:128, 1:2] = W3
    b1s = np.concatenate([b1c, b1c]).reshape(128, 1).astype(np.float32)
    b2s = np.concatenate([b2, b2]).reshape(128, 1).astype(np.float32)

    n1 = feats[ei[0]]  # (E, 2)
    n2 = feats[ei[1]]
    PT = np.ascontiguousarray(
        np.concatenate([n1, n2], axis=1).T
    )  # (4, E) rows: n1f0,n1f1,n2f0,n2f1

    nc = _build_nc()
    in_maps = []
    for c in range(N_CORES):
        ptc = PT[:, c * EDGES_PER_CORE : (c + 1) * EDGES_PER_CORE]
        if EC_PAD != EDGES_PER_CORE:
            ptc = np.concatenate(
                [ptc, np.zeros((4, EC_PAD - EDGES_PER_CORE), np.float32)], axis=1
            )
        # pack: [4, NT, 2, F] -> [2,4,NT,F] -> [8, NT*F]
        pts = np.ascontiguousarray(
            ptc.reshape(4, NT, 2, F).transpose(2, 0, 1, 3).reshape(8, NT * F)
        )
        in_maps.append(
            {"pt": pts, "w1": W1s, "b1": b1s, "w2": W2s, "b2": b2s, "w3": w3s}
        )

    trace = bool(int(os.environ.get("KBENCH_TRACE", "0")))
    import time as _time

    _t0 = _time.time()
    res = bass_utils.run_bass_kernel_spmd(
        nc, in_maps, core_ids=list(range(N_CORES)), trace=trace
    )
    _NC_CACHE["last_exec_wall_ns"] = (_time.time() - _t0) * 1e9
    if trace and res.exec_time_ns is not None:
        _NC_CACHE["last_exec_time_ns"] = res.exec_time_ns

    logits = np.empty(N_EDGES, np.float32)
    for c in range(N_CORES):
        lgc = res.results[c]["lg"].reshape(2, NT, F).transpose(1, 0, 2).ravel()
        logits[c * EDGES_PER_CORE : (c + 1) * EDGES_PER_CORE] = lgc[
            :EDGES_PER_CORE
        ]
    logits = logits + b3[0]

    # softmax over all edges (normalization on host)
    m = logits.max()
    e = np.exp((logits - m).astype(np.float64))
    pi = (e / e.sum()).astype(np.float32).reshape(N_EDGES, 1)
    return pi, value.astype(np.float32)


if __name__ == "__main__":
    # smoke test with random data
    rng = np.random.default_rng(0)
    x = rng.standard_normal((N_NODES, 2), dtype=np.float32)
    ei = rng.integers(0, N_NODES, size=(2, N_EDGES), dtype=np.int64)

    def mk_mlp(dims, bn):
        p = {
            "W": [rng.standard_normal((a, b), dtype=np.float32) / np.sqrt(a)
                  for a, b in zip(dims[:-1], dims[1:])],
            "b": [np.zeros(b, np.float32) for b in dims[1:]],
        }
        if bn:
            p["gamma"] = [np.ones(d, np.float32) for d in dims[1:-1]]
            p["beta"] = [np.zeros(d, np.float32) for d in dims[1:-1]]
        return p

    gin = [mk_mlp([2, 3, 3, 2], True) for _ in range(3)]
    actor = mk_mlp([6, 64, 64, 1], False)
    critic = mk_mlp([2, 64, 64, 1], False)
    pi, v = kernel(x=x, edge_index=ei, gin_params=gin, actor_params=actor,
                   critic_params=critic)
    print("pi", pi.shape, pi.sum(), "value", v)


# revision 11
# speedup vs baseline: 39.4893x; 1.1147x over previous
"""Trainium2 kernel for nn_ActorCritic (GNN message passing + edge-wise actor MLP).

Strategy (per sharding hint): edges sharded across 8 NeuronCores. The
dominant compute -- the per-edge actor MLP (3.2M edges x [4->64->64->1]) --
runs on device as a 2-block-packed PE matmul pipeline (edges stream on the
free dim, two 64-wide blocks packed into the 128-wide PE array). Node
features and the tiny MLP weights are replicated. Graph index prep /
sharding / unshard run on host.
"""

import os
import sys

import numpy as np

sys.path.insert(0, "/opt/trn_rl_repo")

N_NODES = 100000
N_EDGES = 3200000
BN_EPS = 1e-5
N_CORES = 8
F = 512  # matmul free-dim tile (fp32 PSUM bank limit)
EDGES_PER_CORE = N_EDGES // N_CORES  # 400000
NT = (EDGES_PER_CORE + 2 * F - 1) // (2 * F)  # 391 iterations, 1024 edges each
EC_PAD = NT * 2 * F

_NC_CACHE = {}


def _np32(a):
    return np.asarray(a, dtype=np.float32)


def _segment_sum(vals, idx, n):
    # vals [E, 2] f32, idx [E] int64 -> [n, 2] f32
    out = np.empty((n, vals.shape[1]), dtype=np.float32)
    for f in range(vals.shape[1]):
        out[:, f] = np.bincount(idx, weights=vals[:, f], minlength=n).astype(
            np.float32
        )
    return out


def _mlp_host(p, h, bn):
    Ws = [_np32(w) for w in p["W"]]
    bs = [_np32(b) for b in p["b"]]
    n = len(Ws)
    for i in range(n - 1):
        h = h @ Ws[i] + bs[i]
        if bn:
            m = h.mean(axis=0)
            v = h.var(axis=0)
            h = (h - m) / np.sqrt(v + BN_EPS) * _np32(p["gamma"][i]) + _np32(
                p["beta"][i]
            )
        h = np.maximum(h, 0.0)
    return h @ Ws[-1] + bs[-1]


def _build_nc():
    """Build + compile the 8-core SPMD actor-MLP kernel (cached per process)."""
    if "nc" in _NC_CACHE:
        return _NC_CACHE["nc"]
    import concourse.bacc as bacc
    import concourse.tile as tile
    from concourse import bass, mybir

    f32 = mybir.dt.float32
    nc = bacc.Bacc("TRN2", target_bir_lowering=False, debug=False,
                   num_devices=N_CORES)
    pt = nc.dram_tensor("pt", [8, NT * F], f32, kind="ExternalInput")
    w1 = nc.dram_tensor("w1", [8, 128], f32, kind="ExternalInput")
    b1 = nc.dram_tensor("b1", [128, 1], f32, kind="ExternalInput")
    w2 = nc.dram_tensor("w2", [128, 128], f32, kind="ExternalInput")
    b2 = nc.dram_tensor("b2", [128, 1], f32, kind="ExternalInput")
    w3 = nc.dram_tensor("w3", [128, 16 * 32], f32, kind="ExternalInput")
    lg = nc.dram_tensor("lg", [2, NT * F], f32, kind="ExternalOutput")

    CH = 16  # tiles per DMA chunk == tiles per batched-logit PSUM flush
    with tile.TileContext(nc) as tc:
        with (
            tc.tile_pool(name="const", bufs=1) as cpool,
            tc.tile_pool(name="io", bufs=3) as iopool,
            tc.tile_pool(name="act", bufs=3) as apool,
            tc.tile_pool(name="ps1", bufs=2, space="PSUM") as ps1pool,
            tc.tile_pool(name="ps2", bufs=2, space="PSUM") as ps2pool,
            tc.tile_pool(name="ps3", bufs=2, space="PSUM") as ps3pool,
        ):
            w1t = cpool.tile([8, 128], f32)
            nc.sync.dma_start(out=w1t[:], in_=w1.ap())
            b1t = cpool.tile([128, 1], f32)
            nc.sync.dma_start(out=b1t[:], in_=b1.ap())
            w2t = cpool.tile([128, 128], f32)
            nc.sync.dma_start(out=w2t[:], in_=w2.ap())
            b2t = cpool.tile([128, 1], f32)
            nc.sync.dma_start(out=b2t[:], in_=b2.ap())
            w3t = cpool.tile([128, 16 * 32], f32)
            nc.sync.dma_start(out=w3t[:], in_=w3.ap())

            for j0 in range(0, NT, CH):
                n = min(CH, NT - j0)
                rhs = iopool.tile([8, CH * F], f32, tag="rhs")
                nc.sync.dma_start(
                    out=rhs[:, : n * F], in_=pt.ap()[:, j0 * F : (j0 + n) * F]
                )
                ps3g = ps3pool.tile([32, F], f32, tag="ps3")
                for t in range(n):
                    r = rhs[:, t * F : (t + 1) * F]
                    ps1 = ps1pool.tile([128, F], f32, tag="ps1")
                    nc.tensor.matmul(ps1[:], w1t[:], r, start=True, stop=True)
                    h1 = apool.tile([128, F], f32, tag="h1")
                    # h1 = max(ps1 + b1, 0)  (bias-add + relu fused on DVE)
                    nc.vector.tensor_scalar(
                        out=h1[:], in0=ps1[:],
                        scalar1=b1t[:, 0:1], scalar2=0.0,
                        op0=mybir.AluOpType.add, op1=mybir.AluOpType.max,
                    )
                    ps2 = ps2pool.tile([128, F], f32, tag="ps2")
                    nc.tensor.matmul(ps2[:], w2t[:], h1[:], start=True, stop=True)
                    h2 = apool.tile([128, F], f32, tag="h2")
                    nc.scalar.activation(
                        h2[:], ps2[:], mybir.ActivationFunctionType.Relu,
                        bias=b2t[:, 0:1], scale=1.0,
                    )
                    # tile t's w3-block sits at columns 2t,2t+1 of its M=32
                    # weight slice; the 16 matmuls accumulate into one bank
                    # (rows other than 2t,2t+1 contribute exact zeros)
                    nc.tensor.matmul(
                        ps3g[:, :], w3t[:, t * 32 : (t + 1) * 32], h2[:],
                        start=(t == 0), stop=(t == n - 1),
                    )
                lgt = iopool.tile([32, F], f32, tag="lgt")
                nc.vector.tensor_copy(lgt[: 2 * n, :], ps3g[: 2 * n, :])
                # SBUF partition 2t+r -> lg[r, (j0+t)*F : (j0+t+1)*F]
                out_ap = lg.ap()[:, j0 * F : (j0 + n) * F].rearrange(
                    "r (t c) -> t r c", c=F
                )
                nc.sync.dma_start(out=out_ap, in_=lgt[: 2 * n, :])
    nc.compile()
    _NC_CACHE["nc"] = nc
    return nc


def kernel(x, edge_index, gin_params, actor_params, critic_params):
    from concourse import bass_utils

    x = _np32(x)
    ei = np.asarray(edge_index)
    idx_dtype = ei.dtype
    ei = ei.astype(np.int64)
    src = np.concatenate([ei[0], ei[1]])
    dst = np.concatenate([ei[1], ei[0]])

    # ---- GIN stack (tiny 2->3->3->2 MLPs; graph structure prep on host) ----
    feats = x
    for p in gin_params:
        agg = _segment_sum(feats[src], dst, N_NODES)
        feats = _mlp_host(p, feats + agg, bn=True)

    graph_emb = feats.mean(axis=0, keepdims=True).astype(np.float32)  # (1, 2)
    value = _mlp_host(critic_params, graph_emb, bn=False)  # (1, 1)

    # ---- actor MLP on device: logits[e] = MLP(state ++ n1 ++ n2) ----
    W1 = _np32(actor_params["W"][0])  # (6, 64)
    W2 = _np32(actor_params["W"][1])  # (64, 64)
    W3 = _np32(actor_params["W"][2])  # (64, 1)
    b1 = _np32(actor_params["b"][0])
    b2 = _np32(actor_params["b"][1])
    b3 = _np32(actor_params["b"][2])  # constant shift -> softmax-invariant

    W1eff = np.ascontiguousarray(W1[2:6])  # (4, 64)
    b1c = b1 + graph_emb[0] @ W1[0:2]  # (64,)

    # block-diagonal 2x packing into the 128-wide PE array
    W1s = np.zeros((8, 128), np.float32)
    W1s[0:4, 0:64] = W1eff
    W1s[4:8, 64:128] = W1eff
    W2s = np.zeros((128, 128), np.float32)
    W2s[0:64, 0:64] = W2
    W2s[64:128, 64:128] = W2
    w3s = np.zeros((128, 16 * 32), np.float32)
    for t in range(16):
        w3s[0:64, t * 32 + 2 * t : t * 32 + 2 * t + 1] = W3
        w3s[64:128, t * 32 + 2 * t + 1 : t * 32 + 2 * t + 2] = W3
    b1s = np.concatenate([b1c, b1c]).reshape(128, 1).astype(np.float32)
    b2s = np.concatenate([b2, b2]).reshape(128, 1).astype(np.float32)

    n1 = feats[ei[0]]  # (E, 2)
    n2 = feats[ei[1]]
    PT = np.ascontiguousarray(
        np.concatenate([n1, n2], axis=1).T
    )  # (4, E) rows: n1f0,n1f1,n2f0,n2f1

    nc = _build_nc()
    in_maps = []
    for c in range(N_CORES):
        ptc = PT[:, c * EDGES_PER_CORE : (c + 1) * EDGES_PER_CORE]
        if EC_PAD != EDGES_PER_CORE:
            ptc = np.concatenate(
                [ptc, np.zeros((4, EC_PAD - EDGES_PER_CORE), np.float32)], axis=1
            )
        # pack: [4, NT, 2, F] -> [2,4,NT,F] -> [8, NT*F]
        pts = np.ascontiguousarray(
            ptc.reshape(4, NT, 2, F).transpose(2, 0, 1, 3).reshape(8, NT * F)
        )
        in_maps.append(
            {"pt": pts, "w1": W1s, "b1": b1s, "w2": W2s, "b2": b2s, "w3": w3s}
        )

    trace = bool(int(os.environ.get("KBENCH_TRACE", "0")))
    import time as _time

    _t0 = _time.time()
    res = bass_utils.run_bass_kernel_spmd(
        nc, in_maps, core_ids=list(range(N_CORES)), trace=trace
    )
    _NC_CACHE["last_exec_wall_ns"] = (_time.time() - _t0) * 1e9
    if trace and res.exec_time_ns is not None:
        _NC_CACHE["last_exec_time_ns"] = res.exec_time_ns

    logits = np.empty(N_EDGES, np.float32)
    for c in range(N_CORES):
        lgc = res.results[c]["lg"].reshape(2, NT, F).transpose(1, 0, 2).ravel()
        logits[c * EDGES_PER_CORE : (c + 1) * EDGES_PER_CORE] = lgc[
            :EDGES_PER_CORE
        ]
    logits = logits + b3[0]

    # softmax over all edges (normalization on host)
    m = logits.max()
    e = np.exp((logits - m).astype(np.float64))
    pi = (e / e.sum()).astype(np.float32).reshape(N_EDGES, 1)
    return pi, value.astype(np.float32)


if __name__ == "__main__":
    # smoke test with random data
    rng = np.random.default_rng(0)
    x = rng.standard_normal((N_NODES, 2), dtype=np.float32)
    ei = rng.integers(0, N_NODES, size=(2, N_EDGES), dtype=np.int64)

    def mk_mlp(dims, bn):
        p = {
            "W": [rng.standard_normal((a, b), dtype=np.float32) / np.sqrt(a)
                  for a, b in zip(dims[:-1], dims[1:])],
            "b": [np.zeros(b, np.float32) for b in dims[1:]],
        }
        if bn:
            p["gamma"] = [np.ones(d, np.float32) for d in dims[1:-1]]
            p["beta"] = [np.zeros(d, np.float32) for d in dims[1:-1]]
        return p

    gin = [mk_mlp([2, 3, 3, 2], True) for _ in range(3)]
    actor = mk_mlp([6, 64, 64, 1], False)
    critic = mk_mlp([2, 64, 64, 1], False)
    pi, v = kernel(x=x, edge_index=ei, gin_params=gin, actor_params=actor,
                   critic_params=critic)
    print("pi", pi.shape, pi.sum(), "value", v)

    # verify device actor-MLP against a host recompute (pi alone would hide
    # logit errors behind softmax normalization)
    feats = x
    for p in gin:
        agg = _segment_sum(
            feats[np.concatenate([ei[0], ei[1]])],
            np.concatenate([ei[1], ei[0]]),
            N_NODES,
        )
        feats = _mlp_host(p, feats + agg, bn=True)
    ge = feats.mean(axis=0, keepdims=True)
    sp = np.concatenate(
        [np.broadcast_to(ge, (N_EDGES, 2)), feats[ei[0]], feats[ei[1]]], axis=1
    )
    lg_host = _mlp_host(actor, sp.astype(np.float32), bn=False)
    m = lg_host.max()
    eh = np.exp((lg_host - m).astype(np.float64))
    pi_host = (eh / eh.sum()).astype(np.float32)
    err = np.linalg.norm(pi - pi_host) / np.linalg.norm(pi_host)
    print("device-vs-host pi rel err:", err)
    assert err < 1e-3, err
